# revision 15
# baseline (speedup 1.0000x reference)
"""CRF loss kernel, K=4 rank-1 chunked variant (depth 128 instead of 256).

The sequence [0,512] splits into four 128-step chunks.  Each chunk's transfer
operator M_i = prod_t D_t E is numerically rank-1 (E is a small perturbation of
the all-ones matrix; non-dominant directions contract ~64x per step, so over 128
steps the residual is ~1e-80): M_i ~ u_i q^T / n_i with u_i = M_i p,
v_i = M_i^T q, n_i = q^T u_i, for ANY positive seeds p, q.

Chains (each 64 batch cols per group, 2 groups packed on 128 partitions):
  fwd pack [a | u1 | u2]  (192 cols):  x <- ef * (E x)
    a:  one-hot START init, chunk0 ef idx r        -> a_128
    u1: ones init,          chunk1 ef idx 128+r    -> u1 = M1 p
    u2: ones init,          chunk2 ef idx 256+r    -> u2 = M2 p
  bwd pack [v1 | v2 | h2 | h3] (256 cols):  x <- ef * (E^T x + inj)
    v1: seed ef[255] (chunk-end D folded in), ef idx 254-r, final mm -> v1 = M1^T q
    v2: seed ef[383],                         ef idx 382-r, final mm -> v2 = M2^T q
    h2: delta-injected over [256,384):  init 0 + carrier d[383], final inj d[256]
    h3: delta-injected over [384,512]:  init ef511*d512*w + carrier d[511], final inj d[384]
  delta injection rides in wb's repurposed dead rows (tags 0/1) with delta data
  delivered through feats rows, exactly as the 2-chain kernel.

Stitch:  s1 = v1.a_128, n_i = sum(u_i), P21 = v2.u1, H21 = h2.u1, H32 = h3.u2
  (all dots over live tags >= 2);  Q = (s1/n1) * (H21 + H32*P21/n2);
  r = ln Q + CZ*len.
"""

import sys

import numpy as np

sys.path.insert(0, "/opt/trn_rl_repo")

S, B, T = 512, 1024, 64
NCORES = 8
BL = B // NCORES
G = 2
BG = BL // G       # 64
CZ = 4.6875        # bf16-exact
R = 128            # rounds (chunk length)
BLK = 16
NEG = -10000.0
WF_C = 3 * BG      # fwd pack cols = 192
WB_C = 4 * BG      # bwd pack cols = 256

_cache: dict = {}
LAST_EXEC_NS = None


def _build():
    import concourse.bacc as bacc
    import concourse.bass as bass
    import concourse.mybir as mybir
    import concourse.tile as tile

    f32 = mybir.dt.float32
    bf16 = mybir.dt.bfloat16
    AF = mybir.ActivationFunctionType
    ALU = mybir.AluOpType

    nc = bacc.Bacc("TRN2", target_bir_lowering=False, debug=False, enable_asserts=False)

    P128 = G * T

    fpk_d = nc.dram_tensor("fpk", (P128, R, WF_C), bf16, kind="ExternalInput")
    bpk_d = nc.dram_tensor("bpk", (P128, R, WB_C), bf16, kind="ExternalInput")
    binit_d = nc.dram_tensor("binit", (P128, WB_C), bf16, kind="ExternalInput")
    wf_d = nc.dram_tensor("wf16", (P128, P128), bf16, kind="ExternalInput")
    wb_d = nc.dram_tensor("wb16", (P128, P128), bf16, kind="ExternalInput")
    tw_d = nc.dram_tensor("tw", (G, BG), f32, kind="ExternalInput")
    out_d = nc.dram_tensor("out", (G, BG), f32, kind="ExternalOutput")

    with tile.TileContext(nc) as tc:
        with (
            tc.tile_pool(name="const", bufs=1) as cpool,
            tc.tile_pool(name="ffeat", bufs=4) as ffpool,
            tc.tile_pool(name="fef", bufs=4) as fepool,
            tc.tile_pool(name="bfeat", bufs=4) as bfpool,
            tc.tile_pool(name="bef", bufs=4) as bepool,
            tc.tile_pool(name="xf", bufs=3) as xfpool,
            tc.tile_pool(name="xb", bufs=3) as xbpool,
            tc.tile_pool(name="acc", bufs=1) as accpool,
            tc.tile_pool(name="fps", bufs=3, space=bass.MemorySpace.PSUM) as fpsum,
            tc.tile_pool(name="bps", bufs=3, space=bass.MemorySpace.PSUM) as bpsum,
            tc.tile_pool(name="rp", bufs=1, space=bass.MemorySpace.PSUM) as rpool,
        ):
            biasz = cpool.tile([P128, 1], f32, tag="biasz")
            nc.vector.memset(biasz[:], -CZ)
            bias0g = cpool.tile([G, 1], f32, tag="bias0g")
            nc.vector.memset(bias0g[:], 0.0)

            # weights arrive pre-exp'd in bf16 (host does exp in f64)
            wf = cpool.tile([P128, P128], bf16, tag="wf")
            nc.sync.dma_start(wf[:], wf_d[:])
            wb = cpool.tile([P128, P128], bf16, tag="wb")
            nc.sync.dma_start(wb[:], wb_d[:])

            tw = cpool.tile([G, BG], f32, tag="tw")
            nc.sync.dma_start(tw[:], tw_d[:])

            x_b = xbpool.tile([P128, WB_C], bf16, tag="xb")
            nc.sync.dma_start(x_b[:], binit_d[:])

            # live-tag group-sum weights (exclude carrier rows 0,1 per group)
            onesg = cpool.tile([P128, G], bf16, tag="onesg")
            nc.vector.memset(onesg[:], 0.0)
            nc.vector.memset(onesg[0:T, 0:1], 1.0)
            nc.vector.memset(onesg[T : 2 * T, 1:2], 1.0)
            nc.vector.memset(onesg[0:2, 0:1], 0.0)
            nc.vector.memset(onesg[T : T + 2, 1:2], 0.0)

            # fwd state init: a-cols one-hot START, u-cols all ones
            x_f = xfpool.tile([P128, WF_C], bf16, tag="xf")
            nc.vector.memset(x_f[:], 0.0)
            nc.vector.memset(x_f[:, BG:WF_C], 1.0)
            nc.vector.memset(x_f[0:1, 0:BG], 1.0)
            nc.vector.memset(x_f[T : T + 1, 0:BG], 1.0)

            vbfin = None
            for blk in range(R // BLK):
                r0 = blk * BLK
                fbf = ffpool.tile([P128, BLK, WF_C], bf16, tag="fbf")
                nc.sync.dma_start(fbf[:], fpk_d[:, r0 : r0 + BLK, :])
                eff = fepool.tile([P128, BLK, WF_C], bf16, tag="eff")
                nc.scalar.activation(eff[:], fbf[:], AF.Exp, bias=biasz[:])

                fbb = bfpool.tile([P128, BLK, WB_C], bf16, tag="fbb")
                nc.sync.dma_start(fbb[:], bpk_d[:, r0 : r0 + BLK, :])
                efb = bepool.tile([P128, BLK, WB_C], bf16, tag="efb")
                nc.scalar.activation(efb[:], fbb[:], AF.Exp, bias=biasz[:])

                for k in range(BLK):
                    r = r0 + k
                    pmf = fpsum.tile([P128, WF_C], f32, tag="pmf")
                    nc.tensor.matmul(pmf[:], wf[:], x_f[:], start=True, stop=True)
                    xf_new = xfpool.tile([P128, WF_C], bf16, tag="xf")
                    nc.vector.tensor_mul(xf_new[:], pmf[:], eff[:, k, :])
                    x_f = xf_new

                    pmb = bpsum.tile([P128, WB_C], f32, tag="pmb")
                    nc.tensor.matmul(pmb[:], wb[:], x_b[:], start=True, stop=True)
                    if r < R - 1:
                        xb_new = xbpool.tile([P128, WB_C], bf16, tag="xb")
                        nc.vector.tensor_mul(xb_new[:], pmb[:], efb[:, k, :])
                        x_b = xb_new
                    else:
                        vbfin = pmb  # [v1 | v2 | h2 | h3] final (PSUM)

            # ---- stitch ----
            macc = accpool.tile([P128, 6, BG], bf16, tag="macc")
            nc.vector.tensor_mul(macc[:, 0, :], vbfin[:, 0:BG], x_f[:, 0:BG])            # s1 = v1*a
            nc.vector.tensor_mul(macc[:, 1, :], vbfin[:, BG : 2 * BG], x_f[:, BG : 2 * BG])   # P21 = v2*u1
            nc.vector.tensor_mul(macc[:, 2, :], vbfin[:, 2 * BG : 3 * BG], x_f[:, BG : 2 * BG])  # H21 = h2*u1
            nc.vector.tensor_mul(macc[:, 3, :], vbfin[:, 3 * BG : 4 * BG], x_f[:, 2 * BG : 3 * BG])  # H32 = h3*u2
            nc.vector.tensor_copy(macc[:, 4, :], x_f[:, BG : 2 * BG])                    # u1
            nc.vector.tensor_copy(macc[:, 5, :], x_f[:, 2 * BG : 3 * BG])                # u2

            rsum = rpool.tile([G, 6, BG], f32, tag="rsum")
            nc.tensor.matmul(rsum[:], onesg[:], macc[:], start=True, stop=True)
            rsb = accpool.tile([G, 6, BG], f32, tag="rsb")
            nc.vector.tensor_copy(rsb[:], rsum[:])

            # log-domain stitch (no divides): Q = (s1/n1)*(H21 + exp(lnH32+lnP21-lnn2))
            lnall = accpool.tile([G, 6, BG], f32, tag="lnall")
            nc.scalar.activation(lnall[:], rsb[:], AF.Ln, bias=bias0g[:])
            t1 = accpool.tile([G, BG], f32, tag="t1")
            nc.vector.tensor_add(t1[:], lnall[:, 3, :], lnall[:, 1, :])    # lnH32+lnP21
            t2 = accpool.tile([G, BG], f32, tag="t2")
            nc.vector.tensor_sub(t2[:], t1[:], lnall[:, 5, :])             # -lnn2
            t3 = accpool.tile([G, BG], f32, tag="t3")
            nc.scalar.activation(t3[:], t2[:], AF.Exp, bias=bias0g[:])     # H32*P21/n2
            tC = accpool.tile([G, BG], f32, tag="tC")
            nc.vector.tensor_add(tC[:], t3[:], rsb[:, 2, :])               # +H21
            lnC = accpool.tile([G, BG], f32, tag="lnC")
            nc.scalar.activation(lnC[:], tC[:], AF.Ln, bias=bias0g[:])
            t4 = accpool.tile([G, BG], f32, tag="t4")
            nc.vector.tensor_add(t4[:], lnC[:], lnall[:, 0, :])            # +lns1
            t5 = accpool.tile([G, BG], f32, tag="t5")
            nc.vector.tensor_sub(t5[:], t4[:], lnall[:, 4, :])             # -lnn1
            rout = accpool.tile([G, BG], f32, tag="rout")
            nc.vector.tensor_add(rout[:], t5[:], tw[:])
            nc.sync.dma_start(out_d[:], rout[:])

    nc.compile()
    return nc


def _pack_core(x, sl):
    # x: (B_sl..., T) for one time index restricted to core slice -> [128, 64]
    return np.ascontiguousarray(
        x.reshape(G, BG, T).transpose(0, 2, 1).reshape(G * T, BG)
    )


def _prep_inputs(feats, mask, transition):
    import ml_dtypes

    bf = ml_dtypes.bfloat16
    feats = np.asarray(feats, dtype=np.float32)
    mask = np.asarray(mask, dtype=np.float32)
    transition = np.asarray(transition, dtype=np.float32)

    lens = mask.sum(axis=0)
    m_pad = np.concatenate([mask, np.zeros((1, B), np.float32)], axis=0)
    d = np.zeros((S + 1, B), np.float32)
    d[1:] = m_pad[:S] - m_pad[1:]

    wf_log = np.full((G * T, G * T), NEG, np.float32)
    wb_blk = transition.copy()
    wb_blk[0, :] = transition[1, :]
    wb_blk[1, :] = NEG
    wb_blk[:, 0] = NEG
    wb_blk[:, 1] = NEG
    wb_blk[1, 1] = 0.0
    wb_blk[1, 0] = 0.0
    wb_log = np.full((G * T, G * T), NEG, np.float32)
    for g in range(G):
        wf_log[g * T : (g + 1) * T, g * T : (g + 1) * T] = transition.T
        wb_log[g * T : (g + 1) * T, g * T : (g + 1) * T] = wb_blk
    wf16 = np.exp(wf_log.astype(np.float64)).astype(bf)
    wb16 = np.exp(wb_log.astype(np.float64)).astype(bf)

    # two feats variants: v (row0 = no-inject), h (row0 = delta encode); row1 = CZ
    f2v = feats.copy()
    f2v[:, :, 1] = CZ
    f2v[:, :, 0] = NEG
    f2h = feats.copy()
    f2h[:, :, 1] = CZ
    f2h[:, :, 0] = np.where(d[:S] == 1.0, np.float32(CZ), np.float32(NEG))

    w64 = np.exp(transition[1, :].astype(np.float64))
    ef255 = np.exp(feats[255].astype(np.float64) - CZ)   # (B, T)
    ef383 = np.exp(feats[383].astype(np.float64) - CZ)
    ef511 = np.exp(feats[511].astype(np.float64) - CZ)

    rr = np.arange(R)
    fwd_idx = [rr, 128 + rr, 256 + rr]                  # a, u1, u2
    bwd_idx_v = [254 - rr, 382 - rr]   # v1, v2 (r=127 slot unused by the TT)
    bwd_idx_h = [382 - rr, 510 - rr]   # h2, h3

    in_maps = []
    for c in range(NCORES):
        sl = slice(c * BL, (c + 1) * BL)
        # packed per-time views: pv/ph [128, S, 64]
        pv = np.ascontiguousarray(
            f2v[:, sl, :].reshape(S, G, BG, T).transpose(1, 3, 0, 2).reshape(G * T, S, BG)
        )
        ph = np.ascontiguousarray(
            f2h[:, sl, :].reshape(S, G, BG, T).transpose(1, 3, 0, 2).reshape(G * T, S, BG)
        )
        fpk = np.empty((G * T, R, WF_C), np.float32)
        for j, idx in enumerate(fwd_idx):
            fpk[:, :, j * BG : (j + 1) * BG] = pv[:, idx, :]
        bpk = np.empty((G * T, R, WB_C), np.float32)
        for j, idx in enumerate(bwd_idx_v):
            bpk[:, :, j * BG : (j + 1) * BG] = pv[:, idx, :]
        for j, idx in enumerate(bwd_idx_h):
            bpk[:, :, (2 + j) * BG : (3 + j) * BG] = ph[:, idx, :]

        # binit: [v1 | v2 | h2 | h3]
        binit = np.zeros((G * T, WB_C), np.float32)
        e255 = ef255[sl].copy(); e255[:, 0] = 0.0; e255[:, 1] = 1.0
        binit[:, 0:BG] = _pack_core(e255.astype(np.float32), sl)
        e383 = ef383[sl].copy(); e383[:, 0] = 0.0; e383[:, 1] = 1.0
        binit[:, BG : 2 * BG] = _pack_core(e383.astype(np.float32), sl)
        h2i = np.zeros((BL, T), np.float32)
        h2i[:, 0] = d[383, sl]; h2i[:, 1] = 1.0
        binit[:, 2 * BG : 3 * BG] = _pack_core(h2i, sl)
        h3i = (ef511[sl] * d[S, sl][:, None] * w64[None, :]).astype(np.float32)
        h3i[:, 0] = d[S - 1, sl]; h3i[:, 1] = 1.0
        binit[:, 3 * BG : 4 * BG] = _pack_core(h3i, sl)

        in_maps.append(
            {
                "fpk": fpk.astype(bf),
                "bpk": bpk.astype(bf),
                "binit": binit.astype(bf),
                "wf16": wf16,
                "wb16": wb16,
                "tw": np.ascontiguousarray(
                    (CZ * lens[sl]).astype(np.float32).reshape(G, BG)
                ),
            }
        )
    return in_maps


def kernel(feats, mask, transition, trace=False):
    global LAST_EXEC_NS
    if "nc" not in _cache:
        _cache["nc"] = _build()
    nc = _cache["nc"]

    in_maps = _prep_inputs(feats, mask, transition)

    from concourse.bass_utils import run_bass_kernel_spmd

    res = run_bass_kernel_spmd(nc, in_maps, core_ids=list(range(NCORES)), trace=trace)
    LAST_EXEC_NS = res.exec_time_ns
    out = np.concatenate([r["out"].reshape(BL) for r in res.results], axis=0)
    return out.astype(np.float32)
